# revision 10
# baseline (speedup 1.0000x reference)
"""Trainium2 Bass kernel: sharded top-1 multi-head attention lookup.

Problem: 10 heads route to 5 memory banks (prog/stack/local/heap/call).
Per head h: q_h = WQ[h] @ query + bQ[h]; scores = mem @ (WK[h].T @ q_h);
best = argmax(scores); value = WV[h] @ mem[best].

Device strategy (8 NeuronCores, each takes 1/8 of every bank's rows):
  - Stream the mem shard through SBUF in [128 x 1024] fp32 tiles
    (partition r holds 16 consecutive rows).
  - DVE 32x32 block-transpose each tile so d-components land on partitions.
  - TensorE (fp16 inputs, fp32 accumulate) computes per-head
    scores with a block-diagonal stationary matrix; two matmuls accumulate
    the two 32-wide halves of d. Column-group packing puts 4 tiles' scores
    into one full [128, 512] PSUM bank.
  - ScalarE copies score banks to a SBUF buffer; DVE max8/max_index emit
    per-partition top-8 (score, index) candidates.
Host gathers the per-partition candidates, rescores them exactly in fp64,
picks each head's global winner, and computes values/best_scores/best_idx.
The FP22 device scores only need to keep the true argmax inside some
partition's top-8, which holds with overwhelming probability.
"""

import os

import numpy as np

N_CORES = 8
D = 64
ROWS_PER_TILE = 4096  # [128 partitions x 32 rows] per tile
TILE_F = 2048  # 32 rows x 64 floats per partition

# (name, total_rows, n_heads, global_head_offset)
BANKS = [
    ("prog", 262144, 2, 0),
    ("stack", 131072, 3, 2),
    ("local", 131072, 2, 5),
    ("heap", 262144, 2, 7),
    ("call", 32768, 1, 9),
]


def _bank_meta():
    metas = []
    coff = 0
    for name, total, nh, hoff in BANKS:
        rows_pc = total // N_CORES
        nt = rows_pc // ROWS_PER_TILE
        ngroups = (nt + 1) // 2
        parts = 32 * min(4, 2 * nt)
        metas.append(
            dict(
                name=name,
                total=total,
                nh=nh,
                hoff=hoff,
                rows_pc=rows_pc,
                nt=nt,
                ngroups=ngroups,
                parts=parts,
                coff=coff,
            )
        )
        coff += 512 * ngroups
    return metas, coff


BANK_META, SCORE_COLS = _bank_meta()

_PROG = None


def _build_program():
    """Build + compile the Bacc/Tile program once per process."""
    global _PROG
    if _PROG is not None:
        return _PROG
    import concourse.tile as tile
    from concourse import bacc, mybir

    f32 = mybir.dt.float32
    f16 = mybir.dt.float16
    u32 = mybir.dt.uint32

    nc = bacc.Bacc(
        "TRN2", target_bir_lowering=False, debug=False, num_devices=N_CORES
    )

    mem_aps, lhs_aps, mx_aps, ix_aps = {}, {}, {}, {}
    for bm in BANK_META:
        name = bm["name"]
        mem_aps[name] = nc.dram_tensor(
            f"mem_{name}", [bm["nt"], 128, TILE_F], f32, kind="ExternalInput"
        ).ap()
        lhs_aps[name] = nc.dram_tensor(
            f"lhs_{name}", [128, 64], f16, kind="ExternalInput"
        ).ap()
        mx_aps[name] = nc.dram_tensor(
            f"mx_{name}", [bm["parts"], 8], f32, kind="ExternalOutput"
        ).ap()
        ix_aps[name] = nc.dram_tensor(
            f"ix_{name}", [bm["parts"], 8], u32, kind="ExternalOutput"
        ).ap()

    with tile.TileContext(nc) as tc:
        with (
            tc.tile_pool(name="lpool", bufs=8) as lpool,
            tc.tile_pool(name="tpool", bufs=4) as tpool,
            tc.tile_pool(name="rpool", bufs=4) as rpool,
            tc.tile_pool(name="ppool", bufs=6, space="PSUM") as ppool,
            tc.tile_pool(name="cpool", bufs=1) as cpool,
            tc.tile_pool(name="spool", bufs=1) as spool,
            tc.tile_pool(name="opool", bufs=1) as opool,
        ):
            lhs_sb = {}
            for bm in BANK_META:
                name = bm["name"]
                lt = cpool.tile([128, 64], f16, tag=f"lhs_{name}")
                nc.sync.dma_start(lt[:], lhs_aps[name][:])
                lhs_sb[name] = lt

            def emit_argmax(bm, sc):
                name = bm["name"]
                p = bm["parts"]
                mx = opool.tile([p, 8], f32, tag=f"mx_{name}")
                ix = opool.tile([p, 8], u32, tag=f"ix_{name}")
                nc.vector.max(mx[:], sc[0:p, :])
                nc.vector.max_index(ix[:], mx[:], sc[0:p, :])
                nc.sync.dma_start(mx_aps[name][:], mx[:])
                nc.sync.dma_start(ix_aps[name][:], ix[:])

            pending = None
            for bm in BANK_META:
                name = bm["name"]
                ncols = 512 * bm["ngroups"]
                sc = spool.tile([128, ncols], f32, tag=f"scores_{name}")
                for g in range(bm["ngroups"]):
                    tiles_in = min(2, bm["nt"] - 2 * g)
                    ps = ppool.tile([128, 512], f32, tag="ps")
                    for t2 in range(tiles_in):
                        t = g * 2 + t2
                        L = lpool.tile([128, TILE_F], f32, tag="L")
                        nc.sync.dma_start(L[:], mem_aps[name][t])
                        T = tpool.tile([128, TILE_F], f32, tag="T")
                        nc.vector.transpose(T[:], L[:])
                        TR = rpool.tile([128, TILE_F], f16, tag="TR")
                        nc.scalar.copy(TR[:], T[:])
                        Tr = TR[:].rearrange("p (u v i) -> p v u i", v=2, i=32)
                        for uh in range(2):
                            cg = 2 * t2 + uh
                            for v in range(2):
                                nc.tensor.matmul(
                                    ps[32 * cg : 32 * (cg + 1), :],
                                    lhs_sb[name][:, 32 * v : 32 * (v + 1)],
                                    Tr[:, v, 16 * uh : 16 * (uh + 1), :],
                                    start=(v == 0),
                                    stop=(v == 1),
                                    tile_position=(0, 32 * cg),
                                )
                    p = 64 * tiles_in
                    nc.scalar.copy(sc[0:p, 512 * g : 512 * (g + 1)], ps[0:p, :])
                    if g == 0 and pending is not None:
                        emit_argmax(*pending)
                        pending = None
                pending = (bm, sc)
            emit_argmax(*pending)

    nc.compile()
    _PROG = nc
    return nc


def _build_lhs(w32):
    """w32: [nh, 64] key vectors -> [128, 64] stationary, col = v*32 + m."""
    nh = w32.shape[0]
    lhs = np.zeros((128, 64), np.float32)
    for v in range(2):
        for h in range(nh):
            for a in range(4):
                lhs[32 * a : 32 * a + 32, 32 * v + 4 * h + a] = w32[
                    h, 32 * v : 32 * v + 32
                ]
    return lhs


def _run_device(nc, in_maps):
    """Run the program on 8 NeuronCores (or CoreSim with KERNEL_SIM=1)."""
    if os.environ.get("KERNEL_SIM") == "1":
        from concourse.bass_interp import CoreSim

        results = []
        for in_map in in_maps:
            sim = CoreSim(nc, trace=False)
            for k, v in in_map.items():
                sim.tensor(k)[:] = v
            sim.simulate()
            out = {}
            for bm in BANK_META:
                out[f"mx_{bm['name']}"] = np.array(sim.tensor(f"mx_{bm['name']}"))
                out[f"ix_{bm['name']}"] = np.array(sim.tensor(f"ix_{bm['name']}"))
            results.append(out)
        return results
    from concourse import bass_utils

    trace = os.environ.get("KERNEL_TRACE") == "1"
    res = bass_utils.run_bass_kernel_spmd(
        nc, in_maps, core_ids=list(range(N_CORES)), trace=trace
    )
    global _LAST_RESULTS
    _LAST_RESULTS = res
    return res.results


_LAST_RESULTS = None


def kernel(
    query_emb,
    prog_embs,
    stack_embs,
    local_embs,
    heap_embs,
    call_embs,
    WQ,
    bQ,
    WK,
    WV1,
    WV_call,
):
    mems = {
        "prog": np.asarray(prog_embs),
        "stack": np.asarray(stack_embs),
        "local": np.asarray(local_embs),
        "heap": np.asarray(heap_embs),
        "call": np.asarray(call_embs),
    }
    WQ = np.asarray(WQ)
    bQ = np.asarray(bQ)
    WK = np.asarray(WK)
    WV1 = np.asarray(WV1)
    WV_call = np.asarray(WV_call)
    query64 = np.asarray(query_emb).astype(np.float64)

    # Per-head query and effective key vector w[h] = sum_t WK[h,t,:] * q[h,t]
    q64 = np.einsum("htd,d->ht", WQ.astype(np.float64), query64) + bQ.astype(
        np.float64
    )  # [10, 2]
    w64 = np.einsum("htd,ht->hd", WK.astype(np.float64), q64)  # [10, 64]
    w32 = w64.astype(np.float32)

    nc = _build_program()

    # Per-core input maps
    base_map = {}
    for bm in BANK_META:
        name = bm["name"]
        base_map[f"lhs_{name}"] = _build_lhs(
            w32[bm["hoff"] : bm["hoff"] + bm["nh"]]
        ).astype(np.float16)
    in_maps = []
    for c in range(N_CORES):
        m = dict(base_map)
        for bm in BANK_META:
            name = bm["name"]
            rows_pc = bm["rows_pc"]
            shard = mems[name][c * rows_pc : (c + 1) * rows_pc]
            m[f"mem_{name}"] = np.ascontiguousarray(
                shard.reshape(bm["nt"], 128, TILE_F)
            )
        in_maps.append(m)

    results = _run_device(nc, in_maps)

    # Host reduction: decode per-partition top-8 candidates, rescore in fp64.
    best_idx = np.zeros(10, np.int64)
    best_scores = np.zeros(10, np.float64)
    for bm in BANK_META:
        name = bm["name"]
        rows_pc = bm["rows_pc"]
        p = np.arange(bm["parts"])
        cg = p // 32
        t2 = cg // 2
        uh = cg % 2
        h_loc = (p % 32) // 4
        alpha = p % 4
        for h in range(bm["nh"]):
            ghead = bm["hoff"] + h
            sel = h_loc == h
            cand_rows = []
            for c in range(N_CORES):
                ix = results[c][f"ix_{name}"].astype(np.int64)  # [parts, 8]
                q = ix[sel]  # [nsel, 8]
                g = q // 512
                n = q % 512
                u = 16 * uh[sel][:, None] + n // 32
                i = n % 32
                row_loc = (
                    (g * 2 + t2[sel][:, None]) * ROWS_PER_TILE
                    + 1024 * alpha[sel][:, None]
                    + 32 * i
                    + u
                )
                cand_rows.append(c * rows_pc + row_loc.ravel())
            cand = np.unique(np.concatenate(cand_rows))
            s = mems[name][cand].astype(np.float64) @ w64[ghead]
            k = np.argmax(s)
            best_idx[ghead] = cand[k]
            best_scores[ghead] = s[k]

    # Values: WV[h] @ mem[best]
    values = []
    for bm in BANK_META:
        name = bm["name"]
        for h in range(bm["nh"]):
            ghead = bm["hoff"] + h
            wv = (
                WV_call[0].astype(np.float64)
                if name == "call"
                else WV1[ghead].astype(np.float64)
            )
            values.append(wv @ mems[name][best_idx[ghead]].astype(np.float64))
    values = np.concatenate(values).astype(np.float32)

    return (
        values,
        best_scores.astype(np.float32),
        best_idx.astype(np.int32),
    )


# revision 12
# speedup vs baseline: 1.0524x; 1.0524x over previous
"""Trainium2 Bass kernel: sharded top-1 multi-head attention lookup.

Problem: 10 heads route to 5 memory banks (prog/stack/local/heap/call).
Per head h: q_h = WQ[h] @ query + bQ[h]; scores = mem @ (WK[h].T @ q_h);
best = argmax(scores); value = WV[h] @ mem[best].

Device strategy (8 NeuronCores, each takes 1/8 of every bank's rows):
  - Stream the mem shard through SBUF in [128 x 1024] fp32 tiles
    (partition r holds 16 consecutive rows).
  - DVE 32x32 block-transpose each tile so d-components land on partitions.
  - TensorE (fp16 inputs, fp32 accumulate) computes per-head
    scores with a block-diagonal stationary matrix; two matmuls accumulate
    the two 32-wide halves of d. Column-group packing puts 4 tiles' scores
    into one full [128, 512] PSUM bank.
  - ScalarE copies score banks to a SBUF buffer; DVE max8/max_index emit
    per-partition top-8 (score, index) candidates.
Host gathers the per-partition candidates, rescores them exactly in fp64,
picks each head's global winner, and computes values/best_scores/best_idx.
The FP22 device scores only need to keep the true argmax inside some
partition's top-8, which holds with overwhelming probability.
"""

import os

import numpy as np

N_CORES = 8
D = 64
ROWS_PER_TILE = 4096  # [128 partitions x 32 rows] per tile
TILE_F = 2048  # 32 rows x 64 floats per partition

# (name, total_rows, n_heads, global_head_offset)
BANKS = [
    ("prog", 262144, 2, 0),
    ("heap", 262144, 2, 7),
    ("stack", 131072, 3, 2),
    ("local", 131072, 2, 5),
    ("call", 32768, 1, 9),
]


def _bank_meta():
    metas = []
    coff = 0
    for name, total, nh, hoff in BANKS:
        rows_pc = total // N_CORES
        nt = rows_pc // ROWS_PER_TILE
        ngroups = (nt + 1) // 2
        parts = 32 * min(4, 2 * nt)
        metas.append(
            dict(
                name=name,
                total=total,
                nh=nh,
                hoff=hoff,
                rows_pc=rows_pc,
                nt=nt,
                ngroups=ngroups,
                parts=parts,
                coff=coff,
            )
        )
        coff += 512 * ngroups
    return metas, coff


BANK_META, SCORE_COLS = _bank_meta()

_PROG = None


def _build_program():
    """Build + compile the Bacc/Tile program once per process."""
    global _PROG
    if _PROG is not None:
        return _PROG
    import concourse.tile as tile
    from concourse import bacc, mybir

    f32 = mybir.dt.float32
    f16 = mybir.dt.float16
    u32 = mybir.dt.uint32

    nc = bacc.Bacc(
        "TRN2", target_bir_lowering=False, debug=False, num_devices=N_CORES
    )

    mem_aps, lhs_aps, mx_aps, ix_aps = {}, {}, {}, {}
    for bm in BANK_META:
        name = bm["name"]
        mem_aps[name] = nc.dram_tensor(
            f"mem_{name}", [bm["nt"], 128, TILE_F], f32, kind="ExternalInput"
        ).ap()
        lhs_aps[name] = nc.dram_tensor(
            f"lhs_{name}", [128, 64], f16, kind="ExternalInput"
        ).ap()
        mx_aps[name] = nc.dram_tensor(
            f"mx_{name}", [bm["parts"], 8], f32, kind="ExternalOutput"
        ).ap()
        ix_aps[name] = nc.dram_tensor(
            f"ix_{name}", [bm["parts"], 8], u32, kind="ExternalOutput"
        ).ap()

    with tile.TileContext(nc) as tc:
        with (
            tc.tile_pool(name="lpool", bufs=8) as lpool,
            tc.tile_pool(name="tpool", bufs=6) as tpool,
            tc.tile_pool(name="rpool", bufs=6) as rpool,
            tc.tile_pool(name="ppool", bufs=6, space="PSUM") as ppool,
            tc.tile_pool(name="cpool", bufs=1) as cpool,
            tc.tile_pool(name="spool", bufs=1) as spool,
            tc.tile_pool(name="opool", bufs=1) as opool,
        ):
            lhs_sb = {}
            for bm in BANK_META:
                name = bm["name"]
                lt = cpool.tile([128, 64], f16, tag=f"lhs_{name}")
                nc.sync.dma_start(lt[:], lhs_aps[name][:])
                lhs_sb[name] = lt

            def emit_argmax(bm, sc):
                name = bm["name"]
                p = bm["parts"]
                mx = opool.tile([p, 8], f32, tag=f"mx_{name}")
                ix = opool.tile([p, 8], u32, tag=f"ix_{name}")
                nc.vector.max(mx[:], sc[0:p, :])
                nc.vector.max_index(ix[:], mx[:], sc[0:p, :])
                nc.sync.dma_start(mx_aps[name][:], mx[:])
                nc.sync.dma_start(ix_aps[name][:], ix[:])

            pending = []  # [(emit_after_group_counter, bm, sc)]
            gctr = 0
            for bm in BANK_META:
                name = bm["name"]
                ncols = 512 * bm["ngroups"]
                sc = spool.tile([128, ncols], f32, tag=f"scores_{name}")
                for g in range(bm["ngroups"]):
                    tiles_in = min(2, bm["nt"] - 2 * g)
                    ps = ppool.tile([128, 512], f32, tag="ps")
                    for t2 in range(tiles_in):
                        t = g * 2 + t2
                        L = lpool.tile([128, TILE_F], f32, tag="L")
                        nc.sync.dma_start(L[:], mem_aps[name][t])
                        T = tpool.tile([128, TILE_F], f32, tag="T")
                        nc.vector.transpose(T[:], L[:])
                        TR = rpool.tile([128, TILE_F], f16, tag="TR")
                        nc.scalar.copy(TR[:], T[:])
                        Tr = TR[:].rearrange("p (u v i) -> p v u i", v=2, i=32)
                        for uh in range(2):
                            cg = 2 * t2 + uh
                            for v in range(2):
                                nc.tensor.matmul(
                                    ps[32 * cg : 32 * (cg + 1), :],
                                    lhs_sb[name][:, 32 * v : 32 * (v + 1)],
                                    Tr[:, v, 16 * uh : 16 * (uh + 1), :],
                                    start=(v == 0),
                                    stop=(v == 1),
                                    tile_position=(0, 32 * cg),
                                )
                    p = 64 * tiles_in
                    nc.scalar.copy(sc[0:p, 512 * g : 512 * (g + 1)], ps[0:p, :])
                    gctr += 1
                    while pending and pending[0][0] <= gctr:
                        _, pbm, psc = pending.pop(0)
                        emit_argmax(pbm, psc)
                pending.append((gctr + 2, bm, sc))
            while pending:
                _, pbm, psc = pending.pop(0)
                emit_argmax(pbm, psc)

    nc.compile()
    _PROG = nc
    return nc


def _build_lhs(w32):
    """w32: [nh, 64] key vectors -> [128, 64] stationary, col = v*32 + m."""
    nh = w32.shape[0]
    lhs = np.zeros((128, 64), np.float32)
    for v in range(2):
        for h in range(nh):
            for a in range(4):
                lhs[32 * a : 32 * a + 32, 32 * v + 4 * h + a] = w32[
                    h, 32 * v : 32 * v + 32
                ]
    return lhs


def _run_device(nc, in_maps):
    """Run the program on 8 NeuronCores (or CoreSim with KERNEL_SIM=1)."""
    if os.environ.get("KERNEL_SIM") == "1":
        from concourse.bass_interp import CoreSim

        results = []
        for in_map in in_maps:
            sim = CoreSim(nc, trace=False)
            for k, v in in_map.items():
                sim.tensor(k)[:] = v
            sim.simulate()
            out = {}
            for bm in BANK_META:
                out[f"mx_{bm['name']}"] = np.array(sim.tensor(f"mx_{bm['name']}"))
                out[f"ix_{bm['name']}"] = np.array(sim.tensor(f"ix_{bm['name']}"))
            results.append(out)
        return results
    from concourse import bass_utils

    trace = os.environ.get("KERNEL_TRACE") == "1"
    res = bass_utils.run_bass_kernel_spmd(
        nc, in_maps, core_ids=list(range(N_CORES)), trace=trace
    )
    global _LAST_RESULTS
    _LAST_RESULTS = res
    return res.results


_LAST_RESULTS = None


def kernel(
    query_emb,
    prog_embs,
    stack_embs,
    local_embs,
    heap_embs,
    call_embs,
    WQ,
    bQ,
    WK,
    WV1,
    WV_call,
):
    mems = {
        "prog": np.asarray(prog_embs),
        "stack": np.asarray(stack_embs),
        "local": np.asarray(local_embs),
        "heap": np.asarray(heap_embs),
        "call": np.asarray(call_embs),
    }
    WQ = np.asarray(WQ)
    bQ = np.asarray(bQ)
    WK = np.asarray(WK)
    WV1 = np.asarray(WV1)
    WV_call = np.asarray(WV_call)
    query64 = np.asarray(query_emb).astype(np.float64)

    # Per-head query and effective key vector w[h] = sum_t WK[h,t,:] * q[h,t]
    q64 = np.einsum("htd,d->ht", WQ.astype(np.float64), query64) + bQ.astype(
        np.float64
    )  # [10, 2]
    w64 = np.einsum("htd,ht->hd", WK.astype(np.float64), q64)  # [10, 64]
    w32 = w64.astype(np.float32)

    nc = _build_program()

    # Per-core input maps
    base_map = {}
    for bm in BANK_META:
        name = bm["name"]
        base_map[f"lhs_{name}"] = _build_lhs(
            w32[bm["hoff"] : bm["hoff"] + bm["nh"]]
        ).astype(np.float16)
    in_maps = []
    for c in range(N_CORES):
        m = dict(base_map)
        for bm in BANK_META:
            name = bm["name"]
            rows_pc = bm["rows_pc"]
            shard = mems[name][c * rows_pc : (c + 1) * rows_pc]
            m[f"mem_{name}"] = np.ascontiguousarray(
                shard.reshape(bm["nt"], 128, TILE_F)
            )
        in_maps.append(m)

    results = _run_device(nc, in_maps)

    # Host reduction: decode per-partition top-8 candidates, rescore in fp64.
    best_idx = np.zeros(10, np.int64)
    best_scores = np.zeros(10, np.float64)
    for bm in BANK_META:
        name = bm["name"]
        rows_pc = bm["rows_pc"]
        p = np.arange(bm["parts"])
        cg = p // 32
        t2 = cg // 2
        uh = cg % 2
        h_loc = (p % 32) // 4
        alpha = p % 4
        for h in range(bm["nh"]):
            ghead = bm["hoff"] + h
            sel = h_loc == h
            cand_rows = []
            for c in range(N_CORES):
                ix = results[c][f"ix_{name}"].astype(np.int64)  # [parts, 8]
                q = ix[sel]  # [nsel, 8]
                g = q // 512
                n = q % 512
                u = 16 * uh[sel][:, None] + n // 32
                i = n % 32
                row_loc = (
                    (g * 2 + t2[sel][:, None]) * ROWS_PER_TILE
                    + 1024 * alpha[sel][:, None]
                    + 32 * i
                    + u
                )
                cand_rows.append(c * rows_pc + row_loc.ravel())
            cand = np.unique(np.concatenate(cand_rows))
            s = mems[name][cand].astype(np.float64) @ w64[ghead]
            k = np.argmax(s)
            best_idx[ghead] = cand[k]
            best_scores[ghead] = s[k]

    # Values: WV[h] @ mem[best] — in global head order
    values = []
    for bm in sorted(BANK_META, key=lambda b: b["hoff"]):
        name = bm["name"]
        for h in range(bm["nh"]):
            ghead = bm["hoff"] + h
            wv = (
                WV_call[0].astype(np.float64)
                if name == "call"
                else WV1[ghead].astype(np.float64)
            )
            values.append(wv @ mems[name][best_idx[ghead]].astype(np.float64))
    values = np.concatenate(values).astype(np.float32)

    return (
        values,
        best_scores.astype(np.float32),
        best_idx.astype(np.int32),
    )
